# revision 15
# baseline (speedup 1.0000x reference)
"""Trainium2 Bass kernel for PositionalCombinatorOp.

Per (b, n) row the op is a ragged concat along the MO axis:
    out[0:fc]       = first_buf[0:fc]        (first  = subs ? right : left)
    out[fc:fc+sc]   = second_buf[0:sc]       (second = subs ? left  : right)
    out[fc+sc:MO]   = 0
    new_count       = min(left_count + right_count, MO)

Strategy (memory-bound): shard B across the 8 cores (one batch each).
The host packs, per 128-row group, the left and right buffers
interleaved per row plus 64 zero rows:
    comb row (r*128 + which*64 + pos) = (which ? right : left)[r, pos]
    comb rows 16384..16447 = zeros
On device, cheap DVE ops build a per-(row, pos) int16 source-row table
(in-first -> first-buf row, in-second -> second-buf row shifted by fc,
tail -> zero rows).  The table is relaid into dma_gather's wrapped
16-partition layout with one DRAM-scratch round trip, then a single
dma_gather per group pulls all 8192 512-byte rows into an SBUF tile
laid out exactly like the output, which one contiguous 4 MB HWDGE DMA
stores back.  No per-element compute touches the data itself.
"""

from contextlib import ExitStack

import numpy as np

B, N, MO, D = 8, 256, 64, 128
RPC = N                  # rows per core (B sharded across 8 cores)
GROUPS = RPC // 128      # 2 groups of 128 rows
GROWS = 128 * 2 * MO     # data rows per group block in comb (16384)
ZROWS = MO               # zero rows appended per group block
GBASE = GROWS + ZROWS    # comb rows per group block (16448)
NROWS = RPC * MO         # output D-rows per core (16384)
RNDC = 8388608.0         # 2^23; x+RNDC-RNDC == round-half-even for |x|<2^22

_CACHE: dict = {}


def _build_bass():
    import concourse.bacc as bacc
    import concourse.bass as bass
    import concourse.mybir as mybir
    import concourse.tile as tile

    f32 = mybir.dt.float32
    i32 = mybir.dt.int32
    i16 = mybir.dt.int16
    u8 = mybir.dt.uint8
    Alu = mybir.AluOpType

    nc = bacc.Bacc(None, target_bir_lowering=False, debug=False)
    comb = nc.declare_dram_parameter("comb", [GROUPS * GBASE, D], f32, isOutput=False)
    lcnt = nc.declare_dram_parameter("lcnt", [RPC], f32, isOutput=False)
    rcnt = nc.declare_dram_parameter("rcnt", [RPC], f32, isOutput=False)
    subs = nc.declare_dram_parameter("subs", [RPC], i32, isOutput=False)
    outb = nc.declare_dram_parameter("outb", [NROWS, D], f32, isOutput=True)
    outc = nc.declare_dram_parameter("outc", [RPC], f32, isOutput=True)

    with ExitStack() as ctx:
        tc = ctx.enter_context(tile.TileContext(nc))
        cpool = ctx.enter_context(tc.tile_pool(name="const", bufs=1))
        sp = ctx.enter_context(tc.tile_pool(name="small", bufs=2))
        bigp = ctx.enter_context(tc.tile_pool(name="big", bufs=2))
        pp = ctx.enter_context(tc.tile_pool(name="psum", bufs=2, space="PSUM"))

        # --- constants ---
        POSi = cpool.tile([128, MO], i32)
        nc.gpsimd.iota(POSi[:], pattern=[[1, MO]], base=0, channel_multiplier=0)
        POS = cpool.tile([128, MO], f32)
        nc.vector.tensor_copy(POS[:], POSi[:])
        # RB[p] = 128*p: base comb row of local row p within its group block
        RBi = cpool.tile([128, 1], i32)
        nc.gpsimd.iota(RBi[:], pattern=[[0, 1]], base=0, channel_multiplier=2 * MO)
        RB = cpool.tile([128, 1], f32)
        nc.vector.tensor_copy(RB[:], RBi[:])
        # ZROW[p, pos] = GROWS + pos: zero-row index for tail positions
        ZROW = cpool.tile([128, MO], f32)
        nc.vector.tensor_scalar(
            out=ZROW[:], in0=POS[:], scalar1=float(GROWS), scalar2=None, op0=Alu.add
        )
        # CHB[p, k, g] = 64*(g*128+p) + 8*k: outb D-row base of store-chunk k
        CHBi = cpool.tile([128, 8, GROUPS], i32)
        nc.gpsimd.iota(
            CHBi[:], pattern=[[8, 8], [128 * MO, GROUPS]], base=0, channel_multiplier=MO
        )
        CHB = cpool.tile([128, 8, GROUPS], f32)
        nc.vector.tensor_copy(CHB[:], CHBi[:])
        # K8[p, k] = 8*k
        K8i = cpool.tile([128, 8], i32)
        nc.gpsimd.iota(K8i[:], pattern=[[8, 8]], base=0, channel_multiplier=0)
        K8 = cpool.tile([128, 8], f32)
        nc.vector.tensor_copy(K8[:], K8i[:])
        # PERM16[r, q] = (r % 16 == q % 16): permutation for the wrapped
        # relayout matmul (partition digit swap via TensorEngine).
        # (r - q) % 16 == 0, via x - ((x+128)>>4<<4) without a mod op
        PD = cpool.tile([128, 128], i32)
        nc.gpsimd.iota(PD[:], pattern=[[-1, 128]], base=128, channel_multiplier=1)
        PDs = cpool.tile([128, 128], i32)
        nc.vector.tensor_scalar(
            out=PDs[:], in0=PD[:], scalar1=4, scalar2=4,
            op0=Alu.arith_shift_right, op1=Alu.logical_shift_left,
        )
        PERM16 = cpool.tile([128, 128], f32)
        nc.vector.tensor_tensor(out=PERM16[:], in0=PD[:], in1=PDs[:], op=Alu.is_equal)
        # RMASK[r, r1] = (r // 16 == r1)
        RD = cpool.tile([128, 8], i32)
        nc.gpsimd.iota(RD[:], pattern=[[-16, 8]], base=0, channel_multiplier=1)
        RGE = cpool.tile([128, 8], f32)
        nc.vector.tensor_scalar(out=RGE[:], in0=RD[:], scalar1=0, scalar2=None, op0=Alu.is_ge)
        RLT = cpool.tile([128, 8], f32)
        nc.vector.tensor_scalar(out=RLT[:], in0=RD[:], scalar1=16, scalar2=None, op0=Alu.is_lt)
        RMASK = cpool.tile([128, 8], f32)
        nc.vector.tensor_tensor(out=RMASK[:], in0=RGE[:], in1=RLT[:], op=Alu.mult)

        # --- per-row scalars for all groups at once: [128, GROUPS] ---
        lc = cpool.tile([128, GROUPS], f32)
        nc.sync.dma_start(out=lc[:], in_=lcnt[:].rearrange("(g p) -> p g", g=GROUPS))
        rc = cpool.tile([128, GROUPS], f32)
        nc.sync.dma_start(out=rc[:], in_=rcnt[:].rearrange("(g p) -> p g", g=GROUPS))
        sbi = cpool.tile([128, GROUPS], i32)
        nc.sync.dma_start(out=sbi[:], in_=subs[:].rearrange("(g p) -> p g", g=GROUPS))
        sb = cpool.tile([128, GROUPS], f32)
        nc.vector.tensor_copy(sb[:], sbi[:])

        # first = lc + sb*(rc-lc); second = lc + rc - first
        dlr = cpool.tile([128, GROUPS], f32)
        nc.vector.tensor_tensor(out=dlr[:], in0=rc[:], in1=lc[:], op=Alu.subtract)
        nc.vector.tensor_tensor(out=dlr[:], in0=dlr[:], in1=sb[:], op=Alu.mult)
        first = cpool.tile([128, GROUPS], f32)
        nc.vector.tensor_tensor(out=first[:], in0=lc[:], in1=dlr[:], op=Alu.add)
        tot = cpool.tile([128, GROUPS], f32)
        nc.vector.tensor_tensor(out=tot[:], in0=lc[:], in1=rc[:], op=Alu.add)
        second = cpool.tile([128, GROUPS], f32)
        nc.vector.tensor_tensor(out=second[:], in0=tot[:], in1=first[:], op=Alu.subtract)

        # new_count = min(lc+rc, MO)  (float, unrounded)
        ncnt = cpool.tile([128, GROUPS], f32)
        nc.vector.tensor_scalar(
            out=ncnt[:], in0=tot[:], scalar1=float(MO), scalar2=None, op0=Alu.min
        )
        nc.sync.dma_start(out=outc[:].rearrange("(g p) -> p g", g=GROUPS), in_=ncnt[:])

        # fc = max(round_half_even(first), 0); sc likewise (exact in f32)
        fc = cpool.tile([128, GROUPS], f32)
        nc.vector.tensor_scalar(out=fc[:], in0=first[:], scalar1=RNDC, scalar2=None, op0=Alu.add)
        nc.vector.tensor_scalar(out=fc[:], in0=fc[:], scalar1=RNDC, scalar2=None, op0=Alu.subtract)
        nc.vector.tensor_scalar(out=fc[:], in0=fc[:], scalar1=0.0, scalar2=None, op0=Alu.max)
        sc = cpool.tile([128, GROUPS], f32)
        nc.vector.tensor_scalar(out=sc[:], in0=second[:], scalar1=RNDC, scalar2=None, op0=Alu.add)
        nc.vector.tensor_scalar(out=sc[:], in0=sc[:], scalar1=RNDC, scalar2=None, op0=Alu.subtract)
        nc.vector.tensor_scalar(out=sc[:], in0=sc[:], scalar1=0.0, scalar2=None, op0=Alu.max)
        fs = cpool.tile([128, GROUPS], f32)
        nc.vector.tensor_tensor(out=fs[:], in0=fc[:], in1=sc[:], op=Alu.add)

        # s1 = RB + 64*sb  (in-first source base row, group-relative)
        # s2 = RB + 64*(1-sb) - fc  (in-second source base row minus fc)
        sb64 = cpool.tile([128, GROUPS], f32)
        nc.vector.tensor_scalar(
            out=sb64[:], in0=sb[:], scalar1=float(MO), scalar2=None, op0=Alu.mult
        )
        s1 = cpool.tile([128, GROUPS], f32)
        nc.vector.tensor_scalar(
            out=s1[:], in0=sb64[:], scalar1=RB[:, 0:1], scalar2=None, op0=Alu.add
        )
        s2 = cpool.tile([128, GROUPS], f32)
        nc.vector.tensor_tensor(out=s2[:], in0=fc[:], in1=sb64[:], op=Alu.add)
        nc.vector.tensor_scalar(out=s2[:], in0=s2[:], scalar1=-1.0, scalar2=None, op0=Alu.mult)
        nc.vector.tensor_scalar(out=s2[:], in0=s2[:], scalar1=float(MO), scalar2=None, op0=Alu.add)
        nc.vector.tensor_scalar(
            out=s2[:], in0=s2[:], scalar1=RB[:, 0:1], scalar2=None, op0=Alu.add
        )

        # --- per group: index table -> wrapped relayout -> gather -> store ---
        for g in range(GROUPS):
            gg = slice(g, g + 1)
            m1 = sp.tile([128, MO], u8, tag="m1")
            nc.vector.tensor_scalar(
                out=m1[:], in0=POS[:], scalar1=fc[:, gg], scalar2=None, op0=Alu.is_lt
            )
            m2 = sp.tile([128, MO], u8, tag="m2")
            nc.vector.tensor_scalar(
                out=m2[:], in0=POS[:], scalar1=fs[:, gg], scalar2=None, op0=Alu.is_lt
            )
            in2 = sp.tile([128, MO], u8, tag="in2")
            nc.vector.tensor_tensor(out=in2[:], in0=m2[:], in1=m1[:], op=Alu.subtract)
            idx1 = sp.tile([128, MO], f32, tag="idx1")
            nc.vector.tensor_scalar(
                out=idx1[:], in0=POS[:], scalar1=s1[:, gg], scalar2=None, op0=Alu.add
            )
            idx2 = sp.tile([128, MO], f32, tag="idx2")
            nc.vector.tensor_scalar(
                out=idx2[:], in0=POS[:], scalar1=s2[:, gg], scalar2=None, op0=Alu.add
            )
            idxf = sp.tile([128, MO], f32, tag="idxf")
            nc.vector.select(out=idxf[:], mask=m1[:], on_true=idx1[:], on_false=ZROW[:])
            nc.vector.copy_predicated(out=idxf[:], mask=in2[:], data=idx2[:])

            # Relayout into dma_gather's wrapped table layout entirely
            # on-chip: table entry for gather element i = pos*128 + r must
            # land at [i%16, i//16] replicated across the 8 partition groups:
            #   W[q, pos*8 + r1] = idxf[16*r1 + q%16, pos]
            # This partition digit swap is a permutation-gather, done as a
            # matmul: W = PERM16.T @ (idxf[r, pos] * RMASK[r, r1]).  Every
            # output is a single-term sum, so it is exact.
            rhsX = sp.tile([128, MO, 8], f32, tag="rhsX")
            idxf_b = bass.AP(
                idxf[:].tensor, idxf[:].offset, [idxf[:].ap[0], [1, MO], [0, 8]]
            )
            rmask_b = bass.AP(
                RMASK[:].tensor, RMASK[:].offset, [RMASK[:].ap[0], [0, MO], [1, 8]]
            )
            nc.vector.tensor_tensor(out=rhsX[:], in0=idxf_b, in1=rmask_b, op=Alu.mult)
            psumW = pp.tile([128, MO * 8], f32, tag="psumW")
            nc.tensor.matmul(
                out=psumW[:],
                lhsT=PERM16[:],
                rhs=rhsX[:].rearrange("p x r -> p (x r)"),
                start=True,
                stop=True,
            )
            W = sp.tile([128, MO * 8], i16, tag="W")
            nc.vector.tensor_copy(W[:], psumW[:])

            T = bigp.tile([128, MO * D], f32, tag="T")
            nc.gpsimd.dma_gather(
                out_ap=T[:].rearrange("p (c e) -> p c e", e=D),
                in_ap=comb[g * GBASE : (g + 1) * GBASE, :],
                idxs_ap=W[:],
                num_idxs=128 * MO,
                num_idxs_reg=128 * MO,
                elem_size=D,
                single_packet=False,
            )
            # Ragged store: the runner donates pre-zeroed output buffers, so
            # all-zero 8-position chunks need no write.  Store each chunk via
            # a per-partition-offset indirect scatter whose offset is the
            # chunk's outb D-row base when the row still has data there
            # (fs > 8k), else an out-of-bounds sentinel that skips the write.
            MK = sp.tile([128, 8], f32, tag="MK")
            nc.vector.tensor_scalar(
                out=MK[:], in0=K8[:], scalar1=fs[:, gg], scalar2=None, op0=Alu.is_lt
            )
            t1 = sp.tile([128, 8], f32, tag="t1")
            nc.vector.tensor_tensor(out=t1[:], in0=CHB[:, :, g], in1=MK[:], op=Alu.mult)
            t2 = sp.tile([128, 8], f32, tag="t2")
            nc.vector.tensor_scalar(
                out=t2[:], in0=MK[:], scalar1=-99999.0, scalar2=99999.0,
                op0=Alu.mult, op1=Alu.add,
            )
            SIDXf = sp.tile([128, 8], f32, tag="SIDXf")
            nc.vector.tensor_tensor(out=SIDXf[:], in0=t1[:], in1=t2[:], op=Alu.add)
            SIDX = sp.tile([128, 8], i32, tag="SIDX")
            nc.vector.tensor_copy(SIDX[:], SIDXf[:])
            for k in range(8):
                nc.gpsimd.indirect_dma_start(
                    out=outb[:],
                    out_offset=bass.IndirectOffsetOnAxis(ap=SIDX[:, k : k + 1], axis=0),
                    in_=T[:, k * 8 * D : (k + 1) * 8 * D],
                    in_offset=None,
                    bounds_check=NROWS - 1,
                    oob_is_err=False,
                )

    nc.compile()
    return nc


def _get_nc():
    if "nc" not in _CACHE:
        _CACHE["nc"] = _build_bass()
    return _CACHE["nc"]


def _make_in_maps(left_buf, left_count, right_buf, right_count, subs):
    left_buf = np.ascontiguousarray(left_buf, dtype=np.float32)
    right_buf = np.ascontiguousarray(right_buf, dtype=np.float32)
    left_count = np.ascontiguousarray(left_count, dtype=np.float32)
    right_count = np.ascontiguousarray(right_count, dtype=np.float32)
    subs = np.ascontiguousarray(subs).astype(np.int32)
    zeros = np.zeros((ZROWS, D), np.float32)
    in_maps = []
    for c in range(B):
        blocks = []
        for g in range(GROUPS):
            rows = slice(g * 128, (g + 1) * 128)
            blk = np.concatenate([left_buf[c, rows], right_buf[c, rows]], axis=1)
            blocks.append(blk.reshape(128 * 2 * MO, D))
            blocks.append(zeros)
        in_maps.append(
            {
                "comb": np.ascontiguousarray(np.concatenate(blocks)),
                "lcnt": left_count[c],
                "rcnt": right_count[c],
                "subs": subs[c],
            }
        )
    return in_maps


def kernel(left_buf, left_count, right_buf, right_count, subs):
    from concourse.bass_utils import run_bass_kernel_spmd

    nc = _get_nc()
    in_maps = _make_in_maps(left_buf, left_count, right_buf, right_count, subs)
    res = run_bass_kernel_spmd(nc, in_maps, core_ids=list(range(B)))
    new_buffer = np.stack(
        [res.results[c]["outb"].reshape(N, MO, D) for c in range(B)]
    )
    new_count = np.stack([res.results[c]["outc"] for c in range(B)])
    return new_buffer, new_count
